# revision 5
# baseline (speedup 1.0000x reference)
"""Bass/Trainium2 kernel for a single-head causal decoder attention head.

Reference computation (fp32):
    k = x @ Wk; q = x @ Wq; v = x @ Wv            # [B,T,H]
    att = softmax(causal(q k^T / sqrt(H)))        # [B,T,T]
    out = att @ v                                 # [B,T,H]
with B=4, T=4096, C=1024, H=128.

Sharding: 8 cores = 4 batches x 2 query-interleave lanes (j in {0,1}).
Core (b, j) handles q-blocks {(2i+j)*512 : i in 0..3}.  The host hands
each core a *permuted* x^T whose columns are [own-lane blocks | other-
lane blocks], so every core runs one identical instruction stream
(SPMD): Q is projected from the first four 512-col groups only, and
attention group i scans a uniform kv span of 4(i+1) chunks in the own
section plus 4(i+1) chunks in the other section.  Causality reduces to
a lane-independent triangular mask on the diagonal block plus a
per-lane all-0/all-1 mask on the final 4 other-section chunks.

Per-core engine budget (throttled PE ~0.5ns/row):
    PE : K/V/Q projections + V transposes + S + PV + quad-folded sums
    ACT: exp only (40 x [128,1024])
    DVE: causal masks, quad-folds, PSUM->SBUF copies, epilogue
    POOL(gpsimd): pair-folds of P for the softmax denominator
    DMA: x^T striped over the sync/gpsimd(/scalar/vector) queues
         (a single hw queue sustains only ~80 GB/s)

Attention inner loop is software-pipelined one tile ahead (S-matmuls
of tile t+1 issue before PV of tile t) so the exp/mask latency never
stalls the tensor engine; sums matmuls trail one tile behind.
"""

import sys

sys.path.insert(0, "/opt/trn_rl_repo")

import numpy as np
import ml_dtypes

import concourse.mybir as mybir
import concourse.tile as tile
from concourse import bacc
from concourse.alu_op_type import AluOpType
from concourse.masks import make_identity
from concourse.bass_utils import run_bass_kernel_spmd

B, T, C, H = 4, 4096, 1024, 128
NCORES = 8
QG = 512                      # q-group width
NG = 4                        # q-groups per core
CB = C // 128                 # 8 contraction chunks
TGRP = T // QG                # 8 column groups of x^T
SCALE = float(H) ** -0.5

BF16 = mybir.dt.bfloat16
F32 = mybir.dt.float32
NPBF16 = ml_dtypes.bfloat16


def _build_program():
    nc = bacc.Bacc("TRN2", target_bir_lowering=False, debug=False)

    xt = nc.dram_tensor("xt", [C, T], BF16, kind="ExternalInput").ap()
    wk = nc.dram_tensor("wk", [C, H], BF16, kind="ExternalInput").ap()
    wq = nc.dram_tensor("wq", [C, H], BF16, kind="ExternalInput").ap()
    wv = nc.dram_tensor("wv", [C, H], BF16, kind="ExternalInput").ap()
    msk = nc.dram_tensor("msk", [128, 8 * QG], BF16, kind="ExternalInput").ap()
    outT = nc.dram_tensor("outT", [H, NG * QG], BF16, kind="ExternalOutput").ap()

    with tile.TileContext(nc) as tc:
        with (
            tc.tile_pool(name="const", bufs=1) as constp,
            tc.tile_pool(name="kvq", bufs=1) as kvqp,
            tc.tile_pool(name="xin", bufs=3) as xinp,
            tc.tile_pool(name="vtb", bufs=2) as vtbp,
            tc.tile_pool(name="attb", bufs=4) as attp,
            tc.tile_pool(name="foldb", bufs=4) as foldp,
            tc.tile_pool(name="epi", bufs=2) as epip,
            tc.tile_pool(name="pp", bufs=2, space="PSUM") as ppool,
            tc.tile_pool(name="ap", bufs=1, space="PSUM") as apool,
        ):
            # --- persistent SBUF tensors ---
            wks = constp.tile([128, CB * H], BF16, tag="wks")
            wqs = constp.tile([128, CB * H], BF16, tag="wqs")
            wvs = constp.tile([128, CB * H], BF16, tag="wvs")
            masks = constp.tile([128, 8 * QG], BF16, tag="masks")
            ident = constp.tile([128, 128], BF16, tag="ident")

            KT = kvqp.tile([128, T], BF16, tag="KT")
            VV = kvqp.tile([128, T], BF16, tag="VV")
            QT = kvqp.tile([128, NG * QG], BF16, tag="QT")
            ones = kvqp.tile([128, 128], BF16, tag="ones")

            xtr = xt.rearrange("(c p) t -> p c t", p=128)
            wkr = wk.rearrange("(c p) h -> p c h", p=128)
            wqr = wq.rearrange("(c p) h -> p c h", p=128)
            wvr = wv.rearrange("(c p) h -> p c h", p=128)

            # --- staged input DMAs: weights + first x tile land first, the
            # 1 MB x tiles are split across the sync and gpsimd hw queues ---
            nc.scalar.dma_start(wks.rearrange("p (c h) -> p c h", c=CB), wkr)
            xg0 = xinp.tile([128, CB * QG], BF16, tag="xg", bufs=3)
            xg0v = xg0.rearrange("p (c q) -> p c q", c=CB)
            nc.sync.dma_start(xg0v[:, 0:1], xtr[:, 0:1, 0:QG])
            nc.gpsimd.dma_start(xg0v[:, 1:3], xtr[:, 1:3, 0:QG])
            nc.scalar.dma_start(wvs.rearrange("p (c h) -> p c h", c=CB), wvr)
            nc.sync.dma_start(xg0v[:, 3:5], xtr[:, 3:5, 0:QG])
            nc.gpsimd.dma_start(xg0v[:, 5:CB], xtr[:, 5:CB, 0:QG])
            nc.scalar.dma_start(wqs.rearrange("p (c h) -> p c h", c=CB), wqr)
            nc.vector.memset(ones, 1.0)
            make_identity(nc, ident)

            pend_tr = []          # deferred (vtt, tg) transpose work

            def do_transposes():
                while pend_tr:
                    vtt, tg = pend_tr.pop(0)
                    tps = ppool.tile([128, QG], BF16, tag="pps")
                    for tb in range(QG // 128):
                        nc.tensor.transpose(
                            tps[:, tb * 128:(tb + 1) * 128],
                            vtt[:, tb * 128:(tb + 1) * 128],
                            ident,
                        )
                    nc.vector.tensor_copy(VV[:, tg * QG:(tg + 1) * QG], tps)

            def proj(tg, with_q):
                if tg == 0:
                    xg = xg0
                else:
                    xg = xinp.tile([128, CB * QG], BF16, tag="xg", bufs=3)
                    xgv = xg.rearrange("p (c q) -> p c q", c=CB)
                    cols = xtr[:, :, tg * QG:(tg + 1) * QG]
                    nc.sync.dma_start(xgv[:, 0:CB // 2], cols[:, 0:CB // 2])
                    nc.gpsimd.dma_start(xgv[:, CB // 2:CB], cols[:, CB // 2:CB])
                kps = ppool.tile([128, QG], F32, tag="pps")
                for ci in range(CB):
                    nc.tensor.matmul(
                        kps,
                        lhsT=wks[:, ci * H:(ci + 1) * H],
                        rhs=xg[:, ci * QG:(ci + 1) * QG],
                        start=(ci == 0),
                        stop=(ci == CB - 1),
                    )
                nc.vector.tensor_copy(KT[:, tg * QG:(tg + 1) * QG], kps)
                do_transposes()    # previous group's V transposes (inputs ready)
                vps = ppool.tile([128, QG], F32, tag="pps")
                for ci in range(CB):
                    nc.tensor.matmul(
                        vps,
                        lhsT=wvs[:, ci * H:(ci + 1) * H],
                        rhs=xg[:, ci * QG:(ci + 1) * QG],
                        start=(ci == 0),
                        stop=(ci == CB - 1),
                    )
                vtt = vtbp.tile([128, QG], BF16, tag="vtt")
                nc.vector.tensor_copy(vtt, vps)
                pend_tr.append((vtt, tg))
                if with_q:
                    qps = ppool.tile([128, QG], F32, tag="pps")
                    for ci in range(CB):
                        nc.tensor.matmul(
                            qps,
                            lhsT=wqs[:, ci * H:(ci + 1) * H],
                            rhs=xg[:, ci * QG:(ci + 1) * QG],
                            start=(ci == 0),
                            stop=(ci == CB - 1),
                        )
                    nc.vector.tensor_copy(QT[:, tg * QG:(tg + 1) * QG], qps)

            def att(i):
                do_transposes()    # flush V transposes the group reads
                qg = QT[:, i * QG:(i + 1) * QG]
                otps = apool.tile([128, QG], F32, tag="otps", bufs=1)
                smps = apool.tile([128, QG], F32, tag="smps", bufs=1)
                ntiles = 2 * (i + 1)
                # tiles: (sec, tp) pairs in order; chunk base and mask offset
                tiles = []
                for sec in range(2):
                    for tp in range(ntiles):
                        mt = tp - (ntiles - 2)
                        moff = sec * 4 * QG + mt * 2 * QG if mt >= 0 else None
                        tiles.append((16 * sec + 2 * tp, moff))
                ntot = len(tiles)
                sps_t = [None] * ntot
                pt_t = [None] * ntot
                fold_t = [None] * ntot

                def emit_s(t):
                    c0, _ = tiles[t]
                    sps = apool.tile([128, 2 * QG], F32, tag="sps", bufs=2)
                    for h in range(2):
                        nc.tensor.matmul(
                            sps[:, h * QG:(h + 1) * QG],
                            lhsT=KT[:, (c0 + h) * 128:(c0 + h + 1) * 128],
                            rhs=qg,
                            start=True,
                            stop=True,
                        )
                    sps_t[t] = sps

                def emit_exp_mask_fold(t):
                    _, moff = tiles[t]
                    pt = attp.tile([128, 2 * QG], BF16, tag="pt")
                    nc.scalar.activation(
                        pt, sps_t[t], mybir.ActivationFunctionType.Exp,
                        scale=SCALE,
                    )
                    sps_t[t] = None
                    if moff is not None:
                        nc.vector.tensor_tensor(
                            pt, pt, masks[:, moff:moff + 2 * QG],
                            op=AluOpType.mult,
                        )
                    fold = foldp.tile([128, QG], BF16, tag="fold")
                    nc.gpsimd.tensor_tensor(
                        fold, pt[:, 0:QG], pt[:, QG:2 * QG], op=AluOpType.add
                    )
                    pt_t[t] = pt
                    fold_t[t] = fold

                def emit_pv(t):
                    c0, _ = tiles[t]
                    for h in range(2):
                        c = c0 + h
                        nc.tensor.matmul(
                            otps,
                            lhsT=VV[:, c * 128:(c + 1) * 128],
                            rhs=pt_t[t][:, h * QG:(h + 1) * QG],
                            start=(t == 0 and h == 0),
                            stop=(t == ntot - 1 and h == 1),
                        )

                def emit_sums(t):
                    # quad-fold: one sums matmul per pair of tiles
                    ff = foldp.tile([128, QG], BF16, tag="ffold")
                    nc.vector.tensor_tensor(
                        ff, fold_t[t - 1], fold_t[t], op=AluOpType.add
                    )
                    fold_t[t - 1] = fold_t[t] = None
                    nc.tensor.matmul(
                        smps, lhsT=ones, rhs=ff,
                        start=(t == 1), stop=(t == ntot - 1),
                    )

                emit_s(0)
                emit_exp_mask_fold(0)
                for t in range(ntot):
                    if t + 1 < ntot:
                        emit_s(t + 1)
                        emit_exp_mask_fold(t + 1)
                    emit_pv(t)
                    pt_t[t] = None
                    if t % 2 == 1:
                        emit_sums(t)
                rb = epip.tile([128, QG], F32, tag="rb")
                nc.vector.reciprocal_approx_fast(rb, smps)
                ot = epip.tile([128, QG], BF16, tag="ot")
                nc.vector.tensor_tensor(ot, otps, rb, op=AluOpType.mult)
                nc.sync.dma_start(outT[:, i * QG:(i + 1) * QG], ot)

            for tg in range(5):
                proj(tg, with_q=(tg < NG))
            nc.scalar.dma_start(masks, msk)
            att(0)
            for k in range(1, NG):
                proj(4 + k, with_q=False)
                att(k)

    if not nc.is_finalized():
        nc.finalize()
    return nc


_NC_CACHE = None


def _get_program():
    global _NC_CACHE
    if _NC_CACHE is None:
        _NC_CACHE = _build_program()
    return _NC_CACHE


def _make_masks(j: int) -> np.ndarray:
    """Multiplicative mask [128, 4096] for lane j.

    Cols [0, 2048): triangular masks for the 4 chunks of the own-section
    diagonal block (chunk c masked where 128*c + kv > q), lane-independent.
    Cols [2048, 4096): pad mask for the final 4 other-section chunks -
    all-zero for lane 0 (padded block), all-one for lane 1 (real block).
    """
    out = np.empty((128, 8 * QG), np.float32)
    kv = np.arange(128)[:, None]
    q = np.arange(QG)[None, :]
    for c in range(4):
        out[:, c * QG:(c + 1) * QG] = (128 * c + kv <= q)
    out[:, 4 * QG:] = float(j)
    return out.astype(NPBF16)


def _run(inputs: dict, trace: bool = False, trace_kwargs: dict | None = None):
    x = np.asarray(inputs["x"], np.float32)
    Wk = np.asarray(inputs["Wk"], np.float32)
    Wq = np.asarray(inputs["Wq"], np.float32)
    Wv = np.asarray(inputs["Wv"], np.float32)

    nc = _get_program()

    wk16 = Wk.astype(NPBF16)
    wq16 = Wq.astype(NPBF16)
    wv16 = Wv.astype(NPBF16)
    msks = [_make_masks(j) for j in range(2)]

    in_maps = []
    for b in range(B):
        xtb = np.ascontiguousarray(x[b].T).astype(NPBF16)  # [C, T]
        for j in range(2):
            xtp = np.concatenate(
                [xtb[:, (2 * i + j) * QG:(2 * i + j + 1) * QG] for i in range(NG)]
                + [xtb[:, (2 * i + 1 - j) * QG:(2 * i + 2 - j) * QG]
                   for i in range(NG)],
                axis=1,
            )
            in_maps.append(
                {
                    "xt": np.ascontiguousarray(xtp),
                    "wk": wk16,
                    "wq": wq16,
                    "wv": wv16,
                    "msk": msks[j],
                }
            )

    res = run_bass_kernel_spmd(
        nc,
        in_maps,
        core_ids=list(range(NCORES)),
        trace=trace,
        **(trace_kwargs or {}),
    )

    out = np.empty((B, T, H), np.float32)
    for core in range(NCORES):
        b, j = divmod(core, 2)
        oT = np.asarray(res.results[core]["outT"], np.float32)  # [H, NG*QG]
        for i in range(NG):
            g = (2 * i + j) * QG
            out[b, g:g + QG, :] = oT[:, i * QG:(i + 1) * QG].T
    return out, res


def kernel(**inputs) -> np.ndarray:
    out, _ = _run(inputs, trace=False)
    return out


# revision 11
# speedup vs baseline: 1.0440x; 1.0440x over previous
"""Bass/Trainium2 kernel for a single-head causal decoder attention head.

Reference computation (fp32):
    k = x @ Wk; q = x @ Wq; v = x @ Wv            # [B,T,H]
    att = softmax(causal(q k^T / sqrt(H)))        # [B,T,T]
    out = att @ v                                 # [B,T,H]
with B=4, T=4096, C=1024, H=128.

Sharding: 8 cores = 4 batches x 2 query-interleave lanes (j in {0,1}).
Core (b, j) handles q-blocks {(2i+j)*512 : i in 0..3}.  The host hands
each core a *permuted* x^T whose columns are [own-lane blocks | other-
lane blocks], so every core runs one identical instruction stream
(SPMD): Q is projected from the first four 512-col groups only, and
attention group i scans a uniform kv span of 4(i+1) chunks in the own
section plus 4(i+1) chunks in the other section.  Causality reduces to
a lane-independent triangular mask on the own-section diagonal block
plus a per-lane all-0/all-1 scalar on the final 4 other-section chunks.

Per-core engine budget (throttled PE ~0.5ns/row):
    PE : K/V/Q projections + V transposes + S + PV + quad-folded sums
    ACT: exp only (40 x [128,1024])
    DVE: causal masks, pair+quad folds, PSUM->SBUF copies, epilogue
    DMA: x^T halves striped over the gpsimd and vector hw queues (a
         single queue sustains only ~40-110 GB/s); outputs on sync.

Attention inner loop is software-pipelined: S-matmuls run one tile
ahead of PV, and the folded softmax-denominator matmuls trail two
tiles, so exp/mask/fold latency never stalls the tensor engine.
"""

import sys

sys.path.insert(0, "/opt/trn_rl_repo")

import numpy as np
import ml_dtypes

import concourse.mybir as mybir
import concourse.tile as tile
from concourse import bacc
from concourse.alu_op_type import AluOpType
from concourse.masks import make_identity
from concourse.bass_utils import run_bass_kernel_spmd

B, T, C, H = 4, 4096, 1024, 128
NCORES = 8
QG = 512                      # q-group width
NG = 4                        # q-groups per core
CB = C // 128                 # 8 contraction chunks
TGRP = T // QG                # 8 column groups of x^T
SCALE = float(H) ** -0.5

BF16 = mybir.dt.bfloat16
F32 = mybir.dt.float32
NPBF16 = ml_dtypes.bfloat16


def _build_program():
    nc = bacc.Bacc("TRN2", target_bir_lowering=False, debug=False)

    xt = nc.dram_tensor("xt", [C, T], BF16, kind="ExternalInput").ap()
    wk = nc.dram_tensor("wk", [C, H], BF16, kind="ExternalInput").ap()
    wq = nc.dram_tensor("wq", [C, H], BF16, kind="ExternalInput").ap()
    wv = nc.dram_tensor("wv", [C, H], BF16, kind="ExternalInput").ap()
    msk = nc.dram_tensor("msk", [128, 4 * QG], BF16, kind="ExternalInput").ap()
    pad = nc.dram_tensor("pad", [128, 1], F32, kind="ExternalInput").ap()
    outT = nc.dram_tensor("outT", [H, NG * QG], BF16, kind="ExternalOutput").ap()

    with tile.TileContext(nc) as tc:
        with (
            tc.tile_pool(name="const", bufs=1) as constp,
            tc.tile_pool(name="kvq", bufs=1) as kvqp,
            tc.tile_pool(name="xin", bufs=3) as xinp,
            tc.tile_pool(name="vtb", bufs=2) as vtbp,
            tc.tile_pool(name="attb", bufs=4) as attp,
            tc.tile_pool(name="foldb", bufs=6) as foldp,
            tc.tile_pool(name="epi", bufs=2) as epip,
            tc.tile_pool(name="pp", bufs=2, space="PSUM") as ppool,
            tc.tile_pool(name="ap", bufs=1, space="PSUM") as apool,
        ):
            # --- persistent SBUF tensors ---
            wks = constp.tile([128, CB * H], BF16, tag="wks")
            wqs = constp.tile([128, CB * H], BF16, tag="wqs")
            wvs = constp.tile([128, CB * H], BF16, tag="wvs")
            masks = constp.tile([128, 4 * QG], BF16, tag="masks")
            padv = constp.tile([128, 1], F32, tag="padv")
            ident = constp.tile([128, 128], BF16, tag="ident")

            KT = kvqp.tile([128, T], BF16, tag="KT")
            VV = kvqp.tile([128, T], BF16, tag="VV")
            QT = kvqp.tile([128, NG * QG], BF16, tag="QT")
            ones = kvqp.tile([128, 128], BF16, tag="ones")

            xtr = xt.rearrange("(c p) t -> p c t", p=128)
            wkr = wk.rearrange("(c p) h -> p c h", p=128)
            wqr = wq.rearrange("(c p) h -> p c h", p=128)
            wvr = wv.rearrange("(c p) h -> p c h", p=128)

            # --- staged input DMAs.  Weights + first x chunks first; each
            # 1 MB x tile is split across the gpsimd and vector hw queues ---
            nc.scalar.dma_start(wks.rearrange("p (c h) -> p c h", c=CB), wkr)
            xg0 = xinp.tile([128, CB * QG], BF16, tag="xg", bufs=3)
            xg0v = xg0.rearrange("p (c q) -> p c q", c=CB)
            nc.gpsimd.dma_start(xg0v[:, 0:1], xtr[:, 0:1, 0:QG])
            nc.scalar.dma_start(xg0v[:, 4:6], xtr[:, 4:6, 0:QG])
            nc.gpsimd.dma_start(xg0v[:, 1:3], xtr[:, 1:3, 0:QG])
            nc.scalar.dma_start(xg0v[:, 6:CB], xtr[:, 6:CB, 0:QG])
            nc.gpsimd.dma_start(xg0v[:, 3:4], xtr[:, 3:4, 0:QG])
            nc.scalar.dma_start(wvs.rearrange("p (c h) -> p c h", c=CB), wvr)
            nc.scalar.dma_start(wqs.rearrange("p (c h) -> p c h", c=CB), wqr)
            nc.scalar.dma_start(padv, pad)
            nc.vector.memset(ones, 1.0)
            make_identity(nc, ident)

            pend_tr = []          # deferred (vtt, tg) transpose work

            def do_transposes():
                while pend_tr:
                    vtt, tg = pend_tr.pop(0)
                    tps = ppool.tile([128, QG], BF16, tag="pps")
                    for tb in range(QG // 128):
                        nc.tensor.transpose(
                            tps[:, tb * 128:(tb + 1) * 128],
                            vtt[:, tb * 128:(tb + 1) * 128],
                            ident,
                        )
                    nc.vector.tensor_copy(VV[:, tg * QG:(tg + 1) * QG], tps)

            def proj(tg, with_q):
                if tg == 0:
                    xg = xg0
                else:
                    xg = xinp.tile([128, CB * QG], BF16, tag="xg", bufs=3)
                    xgv = xg.rearrange("p (c q) -> p c q", c=CB)
                    cols = xtr[:, :, tg * QG:(tg + 1) * QG]
                    if tg < 3:
                        nc.gpsimd.dma_start(xgv[:, 0:4], cols[:, 0:4])
                        nc.scalar.dma_start(xgv[:, 4:CB], cols[:, 4:CB])
                    else:
                        nc.gpsimd.dma_start(xgv[:, 0:3], cols[:, 0:3])
                        nc.sync.dma_start(xgv[:, 3:5], cols[:, 3:5])
                        nc.scalar.dma_start(xgv[:, 5:CB], cols[:, 5:CB])
                kps = ppool.tile([128, QG], F32, tag="pps")
                for ci in range(CB):
                    nc.tensor.matmul(
                        kps,
                        lhsT=wks[:, ci * H:(ci + 1) * H],
                        rhs=xg[:, ci * QG:(ci + 1) * QG],
                        start=(ci == 0),
                        stop=(ci == CB - 1),
                    )
                nc.vector.tensor_copy(KT[:, tg * QG:(tg + 1) * QG], kps)
                do_transposes()    # previous group's V transposes (inputs ready)
                vps = ppool.tile([128, QG], F32, tag="pps")
                for ci in range(CB):
                    nc.tensor.matmul(
                        vps,
                        lhsT=wvs[:, ci * H:(ci + 1) * H],
                        rhs=xg[:, ci * QG:(ci + 1) * QG],
                        start=(ci == 0),
                        stop=(ci == CB - 1),
                    )
                vtt = vtbp.tile([128, QG], BF16, tag="vtt")
                nc.vector.tensor_copy(vtt, vps)
                pend_tr.append((vtt, tg))
                if with_q:
                    qps = ppool.tile([128, QG], F32, tag="pps")
                    for ci in range(CB):
                        nc.tensor.matmul(
                            qps,
                            lhsT=wqs[:, ci * H:(ci + 1) * H],
                            rhs=xg[:, ci * QG:(ci + 1) * QG],
                            start=(ci == 0),
                            stop=(ci == CB - 1),
                        )
                    nc.vector.tensor_copy(QT[:, tg * QG:(tg + 1) * QG], qps)

            def att(i):
                do_transposes()    # flush V transposes the group reads
                qg = QT[:, i * QG:(i + 1) * QG]
                otps = apool.tile([128, QG], F32, tag="otps", bufs=1)
                smps = apool.tile([128, QG], F32, tag="smps", bufs=1)
                ntiles = 2 * (i + 1)
                # tiles: chunk base; mask kind (None | diag-offset | 'pad')
                tiles = []
                for sec in range(2):
                    for tp in range(ntiles):
                        mt = tp - (ntiles - 2)
                        if mt < 0:
                            mk = None
                        elif sec == 0:
                            mk = mt * 2 * QG
                        else:
                            mk = "pad"
                        tiles.append((16 * sec + 2 * tp, mk))
                ntot = len(tiles)
                sps_t = [None] * ntot
                pt_t = [None] * ntot
                fold_t = [None] * ntot

                def emit_s(t):
                    c0, _ = tiles[t]
                    sps = apool.tile([128, 2 * QG], F32, tag="sps", bufs=2)
                    for h in range(2):
                        nc.tensor.matmul(
                            sps[:, h * QG:(h + 1) * QG],
                            lhsT=KT[:, (c0 + h) * 128:(c0 + h + 1) * 128],
                            rhs=qg,
                            start=True,
                            stop=True,
                        )
                    sps_t[t] = sps

                def emit_exp_mask_fold(t):
                    _, mk = tiles[t]
                    pt = attp.tile([128, 2 * QG], BF16, tag="pt")
                    nc.scalar.activation(
                        pt, sps_t[t], mybir.ActivationFunctionType.Exp,
                        scale=SCALE,
                    )
                    sps_t[t] = None
                    if mk == "pad":
                        nc.vector.tensor_scalar_mul(pt, pt, padv)
                    elif mk is not None:
                        nc.vector.tensor_tensor(
                            pt, pt, masks[:, mk:mk + 2 * QG], op=AluOpType.mult
                        )
                    fold = foldp.tile([128, QG], BF16, tag="fold")
                    nc.vector.tensor_tensor(
                        fold, pt[:, 0:QG], pt[:, QG:2 * QG], op=AluOpType.add
                    )
                    pt_t[t] = pt
                    fold_t[t] = fold

                def emit_pv(t):
                    c0, _ = tiles[t]
                    for h in range(2):
                        c = c0 + h
                        nc.tensor.matmul(
                            otps,
                            lhsT=VV[:, c * 128:(c + 1) * 128],
                            rhs=pt_t[t][:, h * QG:(h + 1) * QG],
                            start=(t == 0 and h == 0),
                            stop=(t == ntot - 1 and h == 1),
                        )
                    pt_t[t] = None

                ffs = []
                nsum = [0]
                NSUM = i + 1

                def emit_sums(t):
                    # oct-fold: pair-fold tiles (t-1, t); every second pair
                    # folds again and feeds one sums matmul per 4 tiles
                    ff = foldp.tile([128, QG], BF16, tag="ffold", bufs=3)
                    nc.vector.tensor_tensor(
                        ff, fold_t[t - 1], fold_t[t], op=AluOpType.add
                    )
                    fold_t[t - 1] = fold_t[t] = None
                    ffs.append(ff)
                    if len(ffs) == 2:
                        fff = foldp.tile([128, QG], BF16, tag="fff", bufs=2)
                        nc.vector.tensor_tensor(
                            fff, ffs[0], ffs[1], op=AluOpType.add
                        )
                        ffs.clear()
                        q = nsum[0]
                        nsum[0] += 1
                        nc.tensor.matmul(
                            smps, lhsT=ones, rhs=fff,
                            start=(q == 0), stop=(q == NSUM - 1),
                        )

                emit_s(0)
                emit_exp_mask_fold(0)
                for t in range(ntot):
                    if t + 1 < ntot:
                        emit_s(t + 1)
                        emit_exp_mask_fold(t + 1)
                    emit_pv(t)
                    if t % 2 == 1 and t >= 3:
                        emit_sums(t - 2)      # trail two tiles
                emit_sums(ntot - 1)
                rb = epip.tile([128, QG], F32, tag="rb")
                nc.vector.reciprocal_approx_fast(rb, smps)
                ot = epip.tile([128, QG], BF16, tag="ot")
                nc.vector.tensor_tensor(ot, otps, rb, op=AluOpType.mult)
                nc.gpsimd.dma_start(outT[:, i * QG:(i + 1) * QG], ot)

            proj(0, with_q=True)
            proj(1, with_q=True)
            nc.scalar.dma_start(masks, msk)
            for tg in range(2, 5):
                proj(tg, with_q=(tg < NG))
            att(0)
            for k in range(1, NG):
                proj(4 + k, with_q=False)
                att(k)

    if not nc.is_finalized():
        nc.finalize()
    return nc


_NC_CACHE = None


def _get_program():
    global _NC_CACHE
    if _NC_CACHE is None:
        _NC_CACHE = _build_program()
    return _NC_CACHE


def _make_masks() -> np.ndarray:
    """Triangular masks [128, 2048] for the 4 chunks of the own-section
    diagonal block (chunk c masked where 128*c + kv > q), lane-independent."""
    out = np.empty((128, 4 * QG), np.float32)
    kv = np.arange(128)[:, None]
    q = np.arange(QG)[None, :]
    for c in range(4):
        out[:, c * QG:(c + 1) * QG] = (128 * c + kv <= q)
    return out.astype(NPBF16)


def _run(inputs: dict, trace: bool = False, trace_kwargs: dict | None = None):
    x = np.asarray(inputs["x"], np.float32)
    Wk = np.asarray(inputs["Wk"], np.float32)
    Wq = np.asarray(inputs["Wq"], np.float32)
    Wv = np.asarray(inputs["Wv"], np.float32)

    nc = _get_program()

    wk16 = Wk.astype(NPBF16)
    wq16 = Wq.astype(NPBF16)
    wv16 = Wv.astype(NPBF16)
    msk = _make_masks()
    pads = [np.full((128, 1), float(j), np.float32) for j in range(2)]

    in_maps = []
    for b in range(B):
        xtb = np.ascontiguousarray(x[b].T).astype(NPBF16)  # [C, T]
        for j in range(2):
            xtp = np.concatenate(
                [xtb[:, (2 * i + j) * QG:(2 * i + j + 1) * QG] for i in range(NG)]
                + [xtb[:, (2 * i + 1 - j) * QG:(2 * i + 2 - j) * QG]
                   for i in range(NG)],
                axis=1,
            )
            in_maps.append(
                {
                    "xt": np.ascontiguousarray(xtp),
                    "wk": wk16,
                    "wq": wq16,
                    "wv": wv16,
                    "msk": msk,
                    "pad": pads[j],
                }
            )

    res = run_bass_kernel_spmd(
        nc,
        in_maps,
        core_ids=list(range(NCORES)),
        trace=trace,
        **(trace_kwargs or {}),
    )

    out = np.empty((B, T, H), np.float32)
    for core in range(NCORES):
        b, j = divmod(core, 2)
        oT = np.asarray(res.results[core]["outT"], np.float32)  # [H, NG*QG]
        for i in range(NG):
            g = (2 * i + j) * QG
            out[b, g:g + QG, :] = oT[:, i * QG:(i + 1) * QG].T
    return out, res


def kernel(**inputs) -> np.ndarray:
    out, _ = _run(inputs, trace=False)
    return out


# revision 12
# speedup vs baseline: 1.2093x; 1.1583x over previous
"""Bass/Trainium2 kernel for a single-head causal decoder attention head.

Reference computation (fp32):
    k = x @ Wk; q = x @ Wq; v = x @ Wv            # [B,T,H]
    att = softmax(causal(q k^T / sqrt(H)))        # [B,T,T]
    out = att @ v                                 # [B,T,H]
with B=4, T=4096, C=1024, H=128.

Sharding: 8 cores = 4 batches x 2 query-interleave lanes (j in {0,1}).
Core (b, j) handles q-blocks {(2i+j)*512 : i in 0..3}.  The host hands
each core a *permuted* x^T whose columns are [own-lane blocks | other-
lane blocks], so every core runs one identical instruction stream
(SPMD): Q is projected from the first four 512-col groups only, and
attention group i scans a uniform kv span of 4(i+1) chunks in the own
section plus 4(i+1) chunks in the other section.  Causality reduces to
a lane-independent triangular mask on the own-section diagonal block
plus a per-lane all-0/all-1 scalar on the final 4 other-section chunks.

Per-core engine budget (throttled PE ~0.5ns/row):
    PE : K/V/Q projections + V transposes + S + PV + quad-folded sums
    ACT: exp only (40 x [128,1024])
    DVE: causal masks, pair+quad folds, PSUM->SBUF copies, epilogue
    DMA: x^T halves striped over the gpsimd and vector hw queues (a
         single queue sustains only ~40-110 GB/s); outputs on sync.

Attention inner loop is software-pipelined: S-matmuls run one tile
ahead of PV, and the folded softmax-denominator matmuls trail two
tiles, so exp/mask/fold latency never stalls the tensor engine.
"""

import sys

sys.path.insert(0, "/opt/trn_rl_repo")

import numpy as np
import ml_dtypes

import concourse.mybir as mybir
import concourse.tile as tile
from concourse import bacc
from concourse.alu_op_type import AluOpType
from concourse.masks import make_identity
from concourse.bass_utils import run_bass_kernel_spmd

B, T, C, H = 4, 4096, 1024, 128
NCORES = 8
QG = 512                      # q-group width
NG = 4                        # q-groups per core
CB = C // 128                 # 8 contraction chunks
TGRP = T // QG                # 8 column groups of x^T
SCALE = float(H) ** -0.5

BF16 = mybir.dt.bfloat16
F32 = mybir.dt.float32
NPBF16 = ml_dtypes.bfloat16


def _build_program():
    nc = bacc.Bacc("TRN2", target_bir_lowering=False, debug=False)

    xt = nc.dram_tensor("xt", [C, T], BF16, kind="ExternalInput").ap()
    wk = nc.dram_tensor("wk", [C, H], BF16, kind="ExternalInput").ap()
    wq = nc.dram_tensor("wq", [C, H], BF16, kind="ExternalInput").ap()
    wv = nc.dram_tensor("wv", [C, H], BF16, kind="ExternalInput").ap()
    msk = nc.dram_tensor("msk", [128, 4 * QG], BF16, kind="ExternalInput").ap()
    pad = nc.dram_tensor("pad", [128, 1], F32, kind="ExternalInput").ap()
    outT = nc.dram_tensor("outT", [H, NG * QG], BF16, kind="ExternalOutput").ap()

    with tile.TileContext(nc) as tc:
        with (
            tc.tile_pool(name="const", bufs=1) as constp,
            tc.tile_pool(name="kvq", bufs=1) as kvqp,
            tc.tile_pool(name="xin", bufs=3) as xinp,
            tc.tile_pool(name="vtb", bufs=2) as vtbp,
            tc.tile_pool(name="attb", bufs=4) as attp,
            tc.tile_pool(name="foldb", bufs=6) as foldp,
            tc.tile_pool(name="epi", bufs=2) as epip,
            tc.tile_pool(name="pp", bufs=2, space="PSUM") as ppool,
            tc.tile_pool(name="ap", bufs=1, space="PSUM") as apool,
        ):
            # --- persistent SBUF tensors ---
            wks = constp.tile([128, CB * H], BF16, tag="wks")
            wqs = constp.tile([128, CB * H], BF16, tag="wqs")
            wvs = constp.tile([128, CB * H], BF16, tag="wvs")
            masks = constp.tile([128, 4 * QG], BF16, tag="masks")
            padv = constp.tile([128, 1], F32, tag="padv")
            ident = constp.tile([128, 128], BF16, tag="ident")

            KT = kvqp.tile([128, T], BF16, tag="KT")
            VV = kvqp.tile([128, T], BF16, tag="VV")
            QT = kvqp.tile([128, NG * QG], BF16, tag="QT")
            ones = kvqp.tile([128, 128], BF16, tag="ones")

            xtr = xt.rearrange("(c p) t -> p c t", p=128)
            wkr = wk.rearrange("(c p) h -> p c h", p=128)
            wqr = wq.rearrange("(c p) h -> p c h", p=128)
            wvr = wv.rearrange("(c p) h -> p c h", p=128)

            # --- staged input DMAs.  Weights + first x chunks first; each
            # 1 MB x tile is split across the gpsimd and vector hw queues ---
            nc.scalar.dma_start(wks.rearrange("p (c h) -> p c h", c=CB), wkr)
            xg0 = xinp.tile([128, CB * QG], BF16, tag="xg", bufs=3)
            xg0v = xg0.rearrange("p (c q) -> p c q", c=CB)
            nc.gpsimd.dma_start(xg0v[:, 0:2], xtr[:, 0:2, 0:QG])
            nc.scalar.dma_start(xg0v[:, 4:CB], xtr[:, 4:CB, 0:QG])
            nc.gpsimd.dma_start(xg0v[:, 2:4], xtr[:, 2:4, 0:QG])
            nc.scalar.dma_start(wvs.rearrange("p (c h) -> p c h", c=CB), wvr)
            nc.scalar.dma_start(wqs.rearrange("p (c h) -> p c h", c=CB), wqr)
            nc.scalar.dma_start(padv, pad)
            nc.vector.memset(ones, 1.0)
            make_identity(nc, ident)

            pend_tr = []          # deferred (vtt, tg) transpose work

            def do_transposes():
                while pend_tr:
                    vtt, tg = pend_tr.pop(0)
                    tps = ppool.tile([128, QG], BF16, tag="pps")
                    for tb in range(QG // 128):
                        nc.tensor.transpose(
                            tps[:, tb * 128:(tb + 1) * 128],
                            vtt[:, tb * 128:(tb + 1) * 128],
                            ident,
                        )
                    nc.vector.tensor_copy(VV[:, tg * QG:(tg + 1) * QG], tps)

            def proj(tg, with_q):
                if tg == 0:
                    xg = xg0
                else:
                    xg = xinp.tile([128, CB * QG], BF16, tag="xg", bufs=3)
                    xgv = xg.rearrange("p (c q) -> p c q", c=CB)
                    cols = xtr[:, :, tg * QG:(tg + 1) * QG]
                    if tg < 3:
                        nc.gpsimd.dma_start(xgv[:, 0:4], cols[:, 0:4])
                        nc.scalar.dma_start(xgv[:, 4:CB], cols[:, 4:CB])
                    else:
                        nc.gpsimd.dma_start(xgv[:, 0:3], cols[:, 0:3])
                        nc.sync.dma_start(xgv[:, 3:5], cols[:, 3:5])
                        nc.scalar.dma_start(xgv[:, 5:CB], cols[:, 5:CB])
                kps = ppool.tile([128, QG], F32, tag="pps")
                for ci in range(CB):
                    nc.tensor.matmul(
                        kps,
                        lhsT=wks[:, ci * H:(ci + 1) * H],
                        rhs=xg[:, ci * QG:(ci + 1) * QG],
                        start=(ci == 0),
                        stop=(ci == CB - 1),
                    )
                nc.vector.tensor_copy(KT[:, tg * QG:(tg + 1) * QG], kps)
                do_transposes()    # previous group's V transposes (inputs ready)
                vps = ppool.tile([128, QG], F32, tag="pps")
                for ci in range(CB):
                    nc.tensor.matmul(
                        vps,
                        lhsT=wvs[:, ci * H:(ci + 1) * H],
                        rhs=xg[:, ci * QG:(ci + 1) * QG],
                        start=(ci == 0),
                        stop=(ci == CB - 1),
                    )
                vtt = vtbp.tile([128, QG], BF16, tag="vtt")
                nc.vector.tensor_copy(vtt, vps)
                pend_tr.append((vtt, tg))
                if with_q:
                    qps = ppool.tile([128, QG], F32, tag="pps")
                    for ci in range(CB):
                        nc.tensor.matmul(
                            qps,
                            lhsT=wqs[:, ci * H:(ci + 1) * H],
                            rhs=xg[:, ci * QG:(ci + 1) * QG],
                            start=(ci == 0),
                            stop=(ci == CB - 1),
                        )
                    nc.vector.tensor_copy(QT[:, tg * QG:(tg + 1) * QG], qps)

            def att(i):
                do_transposes()    # flush V transposes the group reads
                qg = QT[:, i * QG:(i + 1) * QG]
                otps = apool.tile([128, QG], F32, tag="otps", bufs=1)
                smps = apool.tile([128, QG], F32, tag="smps", bufs=1)
                ntiles = 2 * (i + 1)
                # tiles: chunk base; mask kind (None | diag-offset | 'pad')
                tiles = []
                for sec in range(2):
                    for tp in range(ntiles):
                        mt = tp - (ntiles - 2)
                        if mt < 0:
                            mk = None
                        elif sec == 0:
                            mk = mt * 2 * QG
                        else:
                            mk = "pad"
                        tiles.append((16 * sec + 2 * tp, mk))
                ntot = len(tiles)
                sps_t = [None] * ntot
                pt_t = [None] * ntot
                fold_t = [None] * ntot

                def emit_s(t):
                    c0, _ = tiles[t]
                    sps = apool.tile([128, 2 * QG], F32, tag="sps", bufs=2)
                    for h in range(2):
                        nc.tensor.matmul(
                            sps[:, h * QG:(h + 1) * QG],
                            lhsT=KT[:, (c0 + h) * 128:(c0 + h + 1) * 128],
                            rhs=qg,
                            start=True,
                            stop=True,
                        )
                    sps_t[t] = sps

                def emit_exp_mask_fold(t):
                    _, mk = tiles[t]
                    pt = attp.tile([128, 2 * QG], BF16, tag="pt")
                    nc.scalar.activation(
                        pt, sps_t[t], mybir.ActivationFunctionType.Exp,
                        scale=SCALE,
                    )
                    sps_t[t] = None
                    if mk == "pad":
                        nc.vector.tensor_scalar_mul(pt, pt, padv)
                    elif mk is not None:
                        nc.vector.tensor_tensor(
                            pt, pt, masks[:, mk:mk + 2 * QG], op=AluOpType.mult
                        )
                    fold = foldp.tile([128, QG], BF16, tag="fold")
                    nc.vector.tensor_tensor(
                        fold, pt[:, 0:QG], pt[:, QG:2 * QG], op=AluOpType.add
                    )
                    pt_t[t] = pt
                    fold_t[t] = fold

                def emit_pv(t):
                    c0, _ = tiles[t]
                    for h in range(2):
                        c = c0 + h
                        nc.tensor.matmul(
                            otps,
                            lhsT=VV[:, c * 128:(c + 1) * 128],
                            rhs=pt_t[t][:, h * QG:(h + 1) * QG],
                            start=(t == 0 and h == 0),
                            stop=(t == ntot - 1 and h == 1),
                        )
                    pt_t[t] = None

                def emit_sums(t):
                    # quad-fold: one sums matmul per pair of tiles (t-1, t)
                    ff = foldp.tile([128, QG], BF16, tag="ffold", bufs=2)
                    nc.vector.tensor_tensor(
                        ff, fold_t[t - 1], fold_t[t], op=AluOpType.add
                    )
                    fold_t[t - 1] = fold_t[t] = None
                    nc.tensor.matmul(
                        smps, lhsT=ones, rhs=ff,
                        start=(t == 1), stop=(t == ntot - 1),
                    )

                emit_s(0)
                emit_exp_mask_fold(0)
                for t in range(ntot):
                    if t + 1 < ntot:
                        emit_s(t + 1)
                        emit_exp_mask_fold(t + 1)
                    emit_pv(t)
                    if t % 2 == 1 and t >= 3:
                        emit_sums(t - 2)      # trail two tiles
                emit_sums(ntot - 1)
                rb = epip.tile([128, QG], F32, tag="rb")
                nc.vector.reciprocal_approx_fast(rb, smps)
                ot = epip.tile([128, QG], BF16, tag="ot")
                nc.vector.tensor_tensor(ot, otps, rb, op=AluOpType.mult)
                nc.sync.dma_start(outT[:, i * QG:(i + 1) * QG], ot)

            proj(0, with_q=True)
            proj(1, with_q=True)
            nc.scalar.dma_start(masks, msk)
            for tg in range(2, 5):
                proj(tg, with_q=(tg < NG))
            att(0)
            for k in range(1, NG):
                proj(4 + k, with_q=False)
                att(k)

    if not nc.is_finalized():
        nc.finalize()
    return nc


_NC_CACHE = None


def _get_program():
    global _NC_CACHE
    if _NC_CACHE is None:
        _NC_CACHE = _build_program()
    return _NC_CACHE


def _make_masks() -> np.ndarray:
    """Triangular masks [128, 2048] for the 4 chunks of the own-section
    diagonal block (chunk c masked where 128*c + kv > q), lane-independent."""
    out = np.empty((128, 4 * QG), np.float32)
    kv = np.arange(128)[:, None]
    q = np.arange(QG)[None, :]
    for c in range(4):
        out[:, c * QG:(c + 1) * QG] = (128 * c + kv <= q)
    return out.astype(NPBF16)


def _run(inputs: dict, trace: bool = False, trace_kwargs: dict | None = None):
    x = np.asarray(inputs["x"], np.float32)
    Wk = np.asarray(inputs["Wk"], np.float32)
    Wq = np.asarray(inputs["Wq"], np.float32)
    Wv = np.asarray(inputs["Wv"], np.float32)

    nc = _get_program()

    wk16 = Wk.astype(NPBF16)
    wq16 = Wq.astype(NPBF16)
    wv16 = Wv.astype(NPBF16)
    msk = _make_masks()
    pads = [np.full((128, 1), float(j), np.float32) for j in range(2)]

    in_maps = []
    for b in range(B):
        xtb = np.ascontiguousarray(x[b].T).astype(NPBF16)  # [C, T]
        for j in range(2):
            xtp = np.concatenate(
                [xtb[:, (2 * i + j) * QG:(2 * i + j + 1) * QG] for i in range(NG)]
                + [xtb[:, (2 * i + 1 - j) * QG:(2 * i + 2 - j) * QG]
                   for i in range(NG)],
                axis=1,
            )
            in_maps.append(
                {
                    "xt": np.ascontiguousarray(xtp),
                    "wk": wk16,
                    "wq": wq16,
                    "wv": wv16,
                    "msk": msk,
                    "pad": pads[j],
                }
            )

    res = run_bass_kernel_spmd(
        nc,
        in_maps,
        core_ids=list(range(NCORES)),
        trace=trace,
        **(trace_kwargs or {}),
    )

    out = np.empty((B, T, H), np.float32)
    for core in range(NCORES):
        b, j = divmod(core, 2)
        oT = np.asarray(res.results[core]["outT"], np.float32)  # [H, NG*QG]
        for i in range(NG):
            g = (2 * i + j) * QG
            out[b, g:g + QG, :] = oT[:, i * QG:(i + 1) * QG].T
    return out, res


def kernel(**inputs) -> np.ndarray:
    out, _ = _run(inputs, trace=False)
    return out


# revision 14
# speedup vs baseline: 1.2150x; 1.0047x over previous
"""Bass/Trainium2 kernel for a single-head causal decoder attention head.

Reference computation (fp32):
    k = x @ Wk; q = x @ Wq; v = x @ Wv            # [B,T,H]
    att = softmax(causal(q k^T / sqrt(H)))        # [B,T,T]
    out = att @ v                                 # [B,T,H]
with B=4, T=4096, C=1024, H=128.

Sharding: 8 cores = 4 batches x 2 query-interleave lanes (j in {0,1}).
Core (b, j) handles q-blocks {(2i+j)*512 : i in 0..3}.  The host hands
each core a *permuted* x^T whose columns are [own-lane blocks | other-
lane blocks], so every core runs one identical instruction stream
(SPMD): Q is projected from the first four 512-col groups only, and
attention group i scans a uniform kv span of 4(i+1) chunks in the own
section plus 4(i+1) chunks in the other section.  Causality reduces to
a lane-independent triangular mask on the own-section diagonal block
plus a per-lane all-0/all-1 scalar on the final 4 other-section chunks.

Per-core engine budget (throttled PE ~0.5ns/row):
    PE : K/V/Q projections + V transposes + S + PV + quad-folded sums
    ACT: exp only (40 x [128,1024])
    DVE: causal masks, pair+quad folds, PSUM->SBUF copies, epilogue
    DMA: x^T halves striped over the gpsimd/scalar/sync hw queues (a
         single queue sustains only ~40-110 GB/s); outputs on sync.

Attention inner loop is software-pipelined: S-matmuls run one tile
ahead of PV, and the folded softmax-denominator matmuls trail two
tiles, so exp/mask/fold latency never stalls the tensor engine.
"""

import sys

sys.path.insert(0, "/opt/trn_rl_repo")

import numpy as np
import ml_dtypes

import concourse.mybir as mybir
import concourse.tile as tile
from concourse import bacc
from concourse.alu_op_type import AluOpType
from concourse.masks import make_identity
from concourse.bass_utils import run_bass_kernel_spmd

B, T, C, H = 4, 4096, 1024, 128
NCORES = 8
QG = 512                      # q-group width
NG = 4                        # q-groups per core
CB = C // 128                 # 8 contraction chunks
TGRP = T // QG                # 8 column groups of x^T
SCALE = float(H) ** -0.5

BF16 = mybir.dt.bfloat16
F32 = mybir.dt.float32
NPBF16 = ml_dtypes.bfloat16


def _build_program():
    nc = bacc.Bacc("TRN2", target_bir_lowering=False, debug=False)

    xt = nc.dram_tensor("xt", [C, T], BF16, kind="ExternalInput").ap()
    wk = nc.dram_tensor("wk", [C, H], BF16, kind="ExternalInput").ap()
    wq = nc.dram_tensor("wq", [C, H], BF16, kind="ExternalInput").ap()
    wv = nc.dram_tensor("wv", [C, H], BF16, kind="ExternalInput").ap()
    msk = nc.dram_tensor("msk", [128, 4 * QG], BF16, kind="ExternalInput").ap()
    pad = nc.dram_tensor("pad", [128, 1], F32, kind="ExternalInput").ap()
    outT = nc.dram_tensor("outT", [H, NG * QG], BF16, kind="ExternalOutput").ap()

    with tile.TileContext(nc) as tc:
        with (
            tc.tile_pool(name="const", bufs=1) as constp,
            tc.tile_pool(name="kvq", bufs=1) as kvqp,
            tc.tile_pool(name="xin", bufs=3) as xinp,
            tc.tile_pool(name="vtb", bufs=2) as vtbp,
            tc.tile_pool(name="attb", bufs=4) as attp,
            tc.tile_pool(name="foldb", bufs=6) as foldp,
            tc.tile_pool(name="epi", bufs=2) as epip,
            tc.tile_pool(name="pp", bufs=2, space="PSUM") as ppool,
            tc.tile_pool(name="ap", bufs=1, space="PSUM") as apool,
        ):
            # --- persistent SBUF tensors ---
            wks = constp.tile([128, CB * H], BF16, tag="wks")
            wqs = constp.tile([128, CB * H], BF16, tag="wqs")
            wvs = constp.tile([128, CB * H], BF16, tag="wvs")
            masks = constp.tile([128, 4 * QG], BF16, tag="masks")
            padv = constp.tile([128, 1], F32, tag="padv")
            ident = constp.tile([128, 128], BF16, tag="ident")

            KT = kvqp.tile([128, T], BF16, tag="KT")
            VV = kvqp.tile([128, T], BF16, tag="VV")
            QT = kvqp.tile([128, NG * QG], BF16, tag="QT")
            ones = kvqp.tile([128, 128], BF16, tag="ones")

            xtr = xt.rearrange("(c p) t -> p c t", p=128)
            wkr = wk.rearrange("(c p) h -> p c h", p=128)
            wqr = wq.rearrange("(c p) h -> p c h", p=128)
            wvr = wv.rearrange("(c p) h -> p c h", p=128)

            # --- staged input DMAs.  Weights + first x chunks first; each
            # 1 MB x tile is split across the gpsimd and vector hw queues ---
            nc.scalar.dma_start(wks.rearrange("p (c h) -> p c h", c=CB), wkr)
            xg0 = xinp.tile([128, CB * QG], BF16, tag="xg", bufs=3)
            xg0v = xg0.rearrange("p (c q) -> p c q", c=CB)
            nc.gpsimd.dma_start(xg0v[:, 0:2], xtr[:, 0:2, 0:QG])
            nc.scalar.dma_start(xg0v[:, 4:CB], xtr[:, 4:CB, 0:QG])
            nc.gpsimd.dma_start(xg0v[:, 2:4], xtr[:, 2:4, 0:QG])
            nc.scalar.dma_start(wvs.rearrange("p (c h) -> p c h", c=CB), wvr)
            nc.scalar.dma_start(wqs.rearrange("p (c h) -> p c h", c=CB), wqr)
            nc.scalar.dma_start(padv, pad)
            nc.vector.memset(ones, 1.0)
            make_identity(nc, ident)

            pend_tr = []          # deferred (vtt, tg) transpose work

            def do_transposes():
                while pend_tr:
                    vtt, tg = pend_tr.pop(0)
                    tps = ppool.tile([128, QG], BF16, tag="pps")
                    for tb in range(QG // 128):
                        nc.tensor.transpose(
                            tps[:, tb * 128:(tb + 1) * 128],
                            vtt[:, tb * 128:(tb + 1) * 128],
                            ident,
                        )
                    nc.vector.tensor_copy(VV[:, tg * QG:(tg + 1) * QG], tps)

            def proj(tg, with_q):
                if tg == 0:
                    xg = xg0
                else:
                    xg = xinp.tile([128, CB * QG], BF16, tag="xg", bufs=3)
                    xgv = xg.rearrange("p (c q) -> p c q", c=CB)
                    cols = xtr[:, :, tg * QG:(tg + 1) * QG]
                    if tg < 3:
                        nc.gpsimd.dma_start(xgv[:, 0:4], cols[:, 0:4])
                        nc.scalar.dma_start(xgv[:, 4:CB], cols[:, 4:CB])
                    else:
                        nc.gpsimd.dma_start(xgv[:, 0:3], cols[:, 0:3])
                        nc.sync.dma_start(xgv[:, 3:5], cols[:, 3:5])
                        nc.scalar.dma_start(xgv[:, 5:CB], cols[:, 5:CB])
                kps = ppool.tile([128, QG], F32, tag="pps")
                for ci in range(CB):
                    nc.tensor.matmul(
                        kps,
                        lhsT=wks[:, ci * H:(ci + 1) * H],
                        rhs=xg[:, ci * QG:(ci + 1) * QG],
                        start=(ci == 0),
                        stop=(ci == CB - 1),
                    )
                nc.vector.tensor_copy(KT[:, tg * QG:(tg + 1) * QG], kps)
                do_transposes()    # previous group's V transposes (inputs ready)
                vps = ppool.tile([128, QG], F32, tag="pps")
                for ci in range(CB):
                    nc.tensor.matmul(
                        vps,
                        lhsT=wvs[:, ci * H:(ci + 1) * H],
                        rhs=xg[:, ci * QG:(ci + 1) * QG],
                        start=(ci == 0),
                        stop=(ci == CB - 1),
                    )
                vtt = vtbp.tile([128, QG], BF16, tag="vtt")
                nc.vector.tensor_copy(vtt, vps)
                pend_tr.append((vtt, tg))
                if with_q:
                    qps = ppool.tile([128, QG], F32, tag="pps")
                    for ci in range(CB):
                        nc.tensor.matmul(
                            qps,
                            lhsT=wqs[:, ci * H:(ci + 1) * H],
                            rhs=xg[:, ci * QG:(ci + 1) * QG],
                            start=(ci == 0),
                            stop=(ci == CB - 1),
                        )
                    nc.vector.tensor_copy(QT[:, tg * QG:(tg + 1) * QG], qps)

            def att(i):
                do_transposes()    # flush V transposes the group reads
                qg = QT[:, i * QG:(i + 1) * QG]
                otps = apool.tile([128, QG], F32, tag="otps", bufs=1)
                smps = apool.tile([128, QG], F32, tag="smps", bufs=1)
                ntiles = 2 * (i + 1)
                # tiles: chunk base; mask kind (None | diag-offset | 'pad')
                tiles = []
                for sec in range(2):
                    for tp in range(ntiles):
                        mt = tp - (ntiles - 2)
                        if mt < 0:
                            mk = None
                        elif sec == 0:
                            mk = mt * 2 * QG
                        else:
                            mk = "pad"
                        tiles.append((16 * sec + 2 * tp, mk))
                ntot = len(tiles)
                sps_t = [None] * ntot
                pt_t = [None] * ntot
                fold_t = [None] * ntot

                def emit_s(t):
                    c0, _ = tiles[t]
                    sps = apool.tile([128, 2 * QG], F32, tag="sps", bufs=2)
                    for h in range(2):
                        nc.tensor.matmul(
                            sps[:, h * QG:(h + 1) * QG],
                            lhsT=KT[:, (c0 + h) * 128:(c0 + h + 1) * 128],
                            rhs=qg,
                            start=True,
                            stop=True,
                        )
                    sps_t[t] = sps

                def emit_exp_mask_fold(t):
                    _, mk = tiles[t]
                    pt = attp.tile([128, 2 * QG], BF16, tag="pt")
                    nc.scalar.activation(
                        pt, sps_t[t], mybir.ActivationFunctionType.Exp,
                        scale=SCALE,
                    )
                    sps_t[t] = None
                    if mk == "pad":
                        nc.vector.tensor_scalar_mul(pt, pt, padv)
                    elif mk is not None:
                        nc.vector.tensor_tensor(
                            pt, pt, masks[:, mk:mk + 2 * QG], op=AluOpType.mult
                        )
                    fold = foldp.tile([128, QG], BF16, tag="fold")
                    nc.vector.tensor_tensor(
                        fold, pt[:, 0:QG], pt[:, QG:2 * QG], op=AluOpType.add
                    )
                    pt_t[t] = pt
                    fold_t[t] = fold

                def emit_pv(t):
                    c0, _ = tiles[t]
                    for h in range(2):
                        c = c0 + h
                        nc.tensor.matmul(
                            otps,
                            lhsT=VV[:, c * 128:(c + 1) * 128],
                            rhs=pt_t[t][:, h * QG:(h + 1) * QG],
                            start=(t == 0 and h == 0),
                            stop=(t == ntot - 1 and h == 1),
                        )
                    pt_t[t] = None

                ffs = []        # pair-folds awaiting a quad partner
                qmm = []        # quad-folds awaiting their sums matmul
                nsum = [0]
                NSUM = i + 1

                def emit_pair(t):
                    # pair-fold tiles (t-1, t); every 2nd pair quad-folds
                    ff = foldp.tile([128, QG], BF16, tag="ffold", bufs=4)
                    nc.vector.tensor_tensor(
                        ff, fold_t[t - 1], fold_t[t], op=AluOpType.add
                    )
                    fold_t[t - 1] = fold_t[t] = None
                    ffs.append(ff)
                    if len(ffs) == 2:
                        fff = foldp.tile([128, QG], BF16, tag="fff", bufs=3)
                        nc.vector.tensor_tensor(
                            fff, ffs[0], ffs[1], op=AluOpType.add
                        )
                        ffs.clear()
                        qmm.append(fff)

                def flush_sums():
                    while qmm:
                        fff = qmm.pop(0)
                        q = nsum[0]
                        nsum[0] += 1
                        nc.tensor.matmul(
                            smps, lhsT=ones, rhs=fff,
                            start=(q == 0), stop=(q == NSUM - 1),
                        )

                emit_s(0)
                emit_exp_mask_fold(0)
                for t in range(ntot):
                    if t + 1 < ntot:
                        emit_s(t + 1)
                        emit_exp_mask_fold(t + 1)
                    emit_pv(t)
                    if t % 2 == 1 and t >= 3:
                        flush_sums()          # quads trail two more tiles
                        emit_pair(t - 2)      # pairs trail two tiles
                emit_pair(ntot - 1)
                flush_sums()
                HQ = QG // 2
                for hh in range(2):
                    sl = slice(hh * HQ, (hh + 1) * HQ)
                    rb = epip.tile([128, HQ], F32, tag="rb", bufs=2)
                    nc.vector.reciprocal_approx_fast(rb, smps[:, sl])
                    ot = epip.tile([128, HQ], BF16, tag="ot", bufs=2)
                    nc.vector.tensor_tensor(ot, otps[:, sl], rb,
                                            op=AluOpType.mult)
                    dst = outT[:, i * QG + hh * HQ:i * QG + (hh + 1) * HQ]
                    if i == NG - 1:
                        nc.gpsimd.dma_start(dst, ot)
                    else:
                        nc.sync.dma_start(dst, ot)

            proj(0, with_q=True)
            proj(1, with_q=True)
            nc.scalar.dma_start(masks, msk)
            for tg in range(2, 5):
                proj(tg, with_q=(tg < NG))
            att(0)
            for k in range(1, NG):
                proj(4 + k, with_q=False)
                att(k)

    if not nc.is_finalized():
        nc.finalize()
    return nc


_NC_CACHE = None


def _get_program():
    global _NC_CACHE
    if _NC_CACHE is None:
        _NC_CACHE = _build_program()
    return _NC_CACHE


def _make_masks() -> np.ndarray:
    """Triangular masks [128, 2048] for the 4 chunks of the own-section
    diagonal block (chunk c masked where 128*c + kv > q), lane-independent."""
    out = np.empty((128, 4 * QG), np.float32)
    kv = np.arange(128)[:, None]
    q = np.arange(QG)[None, :]
    for c in range(4):
        out[:, c * QG:(c + 1) * QG] = (128 * c + kv <= q)
    return out.astype(NPBF16)


def _run(inputs: dict, trace: bool = False, trace_kwargs: dict | None = None):
    x = np.asarray(inputs["x"], np.float32)
    Wk = np.asarray(inputs["Wk"], np.float32)
    Wq = np.asarray(inputs["Wq"], np.float32)
    Wv = np.asarray(inputs["Wv"], np.float32)

    nc = _get_program()

    wk16 = Wk.astype(NPBF16)
    wq16 = Wq.astype(NPBF16)
    wv16 = Wv.astype(NPBF16)
    msk = _make_masks()
    pads = [np.full((128, 1), float(j), np.float32) for j in range(2)]

    in_maps = []
    for b in range(B):
        xtb = np.ascontiguousarray(x[b].T).astype(NPBF16)  # [C, T]
        for j in range(2):
            xtp = np.concatenate(
                [xtb[:, (2 * i + j) * QG:(2 * i + j + 1) * QG] for i in range(NG)]
                + [xtb[:, (2 * i + 1 - j) * QG:(2 * i + 2 - j) * QG]
                   for i in range(NG)],
                axis=1,
            )
            in_maps.append(
                {
                    "xt": np.ascontiguousarray(xtp),
                    "wk": wk16,
                    "wq": wq16,
                    "wv": wv16,
                    "msk": msk,
                    "pad": pads[j],
                }
            )

    res = run_bass_kernel_spmd(
        nc,
        in_maps,
        core_ids=list(range(NCORES)),
        trace=trace,
        **(trace_kwargs or {}),
    )

    out = np.empty((B, T, H), np.float32)
    for core in range(NCORES):
        b, j = divmod(core, 2)
        oT = np.asarray(res.results[core]["outT"], np.float32)  # [H, NG*QG]
        for i in range(NG):
            g = (2 * i + j) * QG
            out[b, g:g + QG, :] = oT[:, i * QG:(i + 1) * QG].T
    return out, res


def kernel(**inputs) -> np.ndarray:
    out, _ = _run(inputs, trace=False)
    return out
